# revision 9
# baseline (speedup 1.0000x reference)
"""Trainium2 Bass kernel for nn_Brain RNN cell (data-parallel over 8 NeuronCores).

Computes, matching the jax reference:
    pre_act  = x @ W_in.T + h @ W_hh.T + b_h          [B, H]
    h_new    = relu(pre_act)                          [B, H]
    act_stats = mean(|h_new|, axis=0)                 [H]
    rec_stats = mean(|h| > 0.1, axis=0)               [H]
    y        = h_new @ W_out.T + b_out                [B, O]

Sharding: batch (8192) split 8 ways; weights replicated. Per-neuron stats are
computed as per-shard partial sums on device and combined into the batch mean
on the host during unsharding.

Device layout: everything feature-major ([feature, batch]) so all matmul
contractions sit on the SBUF partition dim. The host prepares transposed /
block-reordered views of the inputs while sharding, and transposes the
per-core outputs back while gathering; matmuls run as float32r (full PE rate).
"""

import sys

sys.path.insert(0, "/opt/trn_rl_repo")

from contextlib import ExitStack

import numpy as np

import concourse.bass as bass
import concourse.tile as tile
from concourse import bacc, mybir
from concourse.bass_utils import run_bass_kernel_spmd

NCORES = 8
B, I, H, O = 8192, 1024, 2048, 1024
BS = B // NCORES  # per-core batch: 1024
PB = 128  # SBUF partitions
NI, NH, NO = I // PB, H // PB, O // PB  # 8, 16, 8 tiles
NB = 512  # matmul moving-dim chunk (1 PSUM bank of fp32)
NBC = BS // NB  # 2 chunks per core-batch

F32 = mybir.dt.float32
F32R = mybir.dt.float32r
RELU = mybir.ActivationFunctionType.Relu
IDENT = mybir.ActivationFunctionType.Identity
IS_GT = mybir.AluOpType.is_gt
IS_LT = mybir.AluOpType.is_lt
ADD = mybir.AluOpType.add

REC_THRESH = 0.1


def build_bass():
    nc = bacc.Bacc(trn_type="TRN2", target_bir_lowering=False, debug=False)

    # Inputs (per-core shard layouts prepared by the host):
    #   xT[i, b]   = x[b, i]         (shard)        [I, BS]
    #   hT[m, b]   = h[b, m]         (shard)        [H, BS]
    #   w_in[jt, p, it, jj]  = W_in[jt*128+jj, it*128+p]    [NH, PB, NI, PB]
    #   w_hh[jt, p, mt, jj]  = W_hh[jt*128+jj, mt*128+p]    [NH, PB, NH, PB]
    #   w_out[ot, p, jt, oo] = W_out[ot*128+oo, jt*128+p]   [NO, PB, NH, PB]
    #   bh[p, jt] = b_h[jt*128+p]                   [PB, NH]
    #   bo[p, ot] = b_out[ot*128+p]                 [PB, NO]
    xT = nc.dram_tensor("xT", [I, BS], F32R, kind="ExternalInput").ap()
    hT = nc.dram_tensor("hT", [H, BS], F32R, kind="ExternalInput").ap()
    w_in = nc.dram_tensor("w_in", [NH, PB, NI, PB], F32R, kind="ExternalInput").ap()
    w_hh = nc.dram_tensor("w_hh", [NH, PB, NH, PB], F32R, kind="ExternalInput").ap()
    w_out = nc.dram_tensor("w_out", [NO, PB, NH, PB], F32R, kind="ExternalInput").ap()
    bh = nc.dram_tensor("bh", [PB, NH], F32, kind="ExternalInput").ap()
    bo = nc.dram_tensor("bo", [PB, NO], F32, kind="ExternalInput").ap()

    # Outputs: transposed activations + per-partition stat partials.
    #   hnT[j, b] = h_new[b, j]; yT[o, b] = y[b, o]
    #   stats[:, 0:32]  = act partial sums, col jt*NBC+bn
    #   stats[:, 32:64] = rec partial counts, col mt*NBC+bn
    hnT = nc.dram_tensor("hnT", [H, BS], F32R, kind="ExternalOutput").ap()
    yT = nc.dram_tensor("yT", [O, BS], F32, kind="ExternalOutput").ap()
    stats = nc.dram_tensor("stats", [PB, 3 * NH * NBC], F32, kind="ExternalOutput").ap()

    with ExitStack() as ctx:
        tc = ctx.enter_context(tile.TileContext(nc))
        consts = ctx.enter_context(tc.tile_pool(name="consts", bufs=1))
        acts = ctx.enter_context(tc.tile_pool(name="acts", bufs=1))
        wts = ctx.enter_context(tc.tile_pool(name="wts", bufs=2))
        pp = ctx.enter_context(tc.tile_pool(name="pp", bufs=6, space="PSUM"))
        tmps = ctx.enter_context(tc.tile_pool(name="tmps", bufs=3))
        statp = ctx.enter_context(tc.tile_pool(name="statp", bufs=1))

        bh_sb = consts.tile([PB, NH], F32)
        nc.sync.dma_start(bh_sb[:], bh[:])
        bo_sb = consts.tile([PB, NO], F32)
        nc.sync.dma_start(bo_sb[:], bo[:])

        # Resident activations, feature-major: [p, tile, b]
        x_sb = acts.tile([PB, NI, BS], F32R, tag="x")
        nc.sync.dma_start(x_sb[:], xT.rearrange("(it p) b -> p it b", p=PB))
        h_sb = acts.tile([PB, NH, BS], F32R, tag="h")
        nc.sync.dma_start(h_sb[:], hT.rearrange("(mt p) b -> p mt b", p=PB))
        hn_sb = acts.tile([PB, NH, BS], F32R, tag="hn")

        act_acc = statp.tile([PB, NH * NBC], F32)
        rec_gt = statp.tile([PB, NH * NBC], F32)
        rec_lt = statp.tile([PB, NH * NBC], F32)


        # rec_stats partials: count(|h| > 0.1) = count(h > 0.1) + count(h < -0.1),
        # both compared exactly in fp32 on the DVE with a fused add-reduction.
        for mt in range(NH):
            for bn in range(NBC):
                src = h_sb[:, mt, bn * NB : (bn + 1) * NB].bitcast(F32)
                col = mt * NBC + bn
                tmp = tmps.tile([PB, NB], F32, tag="t512")
                nc.vector.tensor_scalar(
                    tmp[:], src, REC_THRESH, None, IS_GT, ADD,
                    accum_out=rec_gt[:, col : col + 1],
                )
                tmp2 = tmps.tile([PB, NB], F32, tag="t512")
                nc.vector.tensor_scalar(
                    tmp2[:], src, -REC_THRESH, None, IS_LT, ADD,
                    accum_out=rec_lt[:, col : col + 1],
                )

        # Stage A: pre_act.T = W_in @ x.T + W_hh @ h.T (+ b_h), relu -> hn_sb
        for jt in range(NH):
            wi = wts.tile([PB, NI, PB], F32R, tag="wA")
            nc.sync.dma_start(wi[:], w_in[jt])
            wh = wts.tile([PB, NH, PB], F32R, tag="wB")
            nc.sync.dma_start(wh[:], w_hh[jt])
            for bn in range(NBC):
                ps = pp.tile([PB, NB], F32, tag="ps")
                for it in range(NI):
                    nc.tensor.matmul(
                        ps[:],
                        lhsT=wi[:, it, :],
                        rhs=x_sb[:, it, bn * NB : (bn + 1) * NB],
                        start=(it == 0),
                        stop=False,
                    )
                for mt in range(NH):
                    nc.tensor.matmul(
                        ps[:],
                        lhsT=wh[:, mt, :],
                        rhs=h_sb[:, mt, bn * NB : (bn + 1) * NB],
                        start=False,
                        stop=(mt == NH - 1),
                    )
                dst = hn_sb[:, jt, bn * NB : (bn + 1) * NB]
                col = jt * NBC + bn
                nc.scalar.activation(
                    dst,
                    ps[:],
                    RELU,
                    bias=bh_sb[:, jt : jt + 1],
                    accum_out=act_acc[:, col : col + 1],
                )
                nc.scalar.dma_start(
                    hnT[jt * PB : (jt + 1) * PB, bn * NB : (bn + 1) * NB], dst
                )

        # Stage B: y.T = W_out @ h_new.T (+ b_out)
        for ot in range(NO):
            wo = wts.tile([PB, NH, PB], F32R, tag="wB")
            nc.sync.dma_start(wo[:], w_out[ot])
            for bn in range(NBC):
                ps = pp.tile([PB, NB], F32, tag="ps")
                for jt in range(NH):
                    nc.tensor.matmul(
                        ps[:],
                        lhsT=wo[:, jt, :],
                        rhs=hn_sb[:, jt, bn * NB : (bn + 1) * NB],
                        start=(jt == 0),
                        stop=(jt == NH - 1),
                    )
                ysb = tmps.tile([PB, NB], F32, tag="y512", bufs=2)
                nc.scalar.activation(ysb[:], ps[:], IDENT, bias=bo_sb[:, ot : ot + 1])
                nc.scalar.dma_start(
                    yT[ot * PB : (ot + 1) * PB, bn * NB : (bn + 1) * NB], ysb[:]
                )

        nc.sync.dma_start(stats[:, 0 : NH * NBC], act_acc[:])
        nc.sync.dma_start(stats[:, NH * NBC : 2 * NH * NBC], rec_gt[:])
        nc.sync.dma_start(stats[:, 2 * NH * NBC :], rec_lt[:])

    nc.compile()
    return nc


_NC_CACHE = {}


def _get_nc():
    if "nc" not in _NC_CACHE:
        _NC_CACHE["nc"] = build_bass()
    return _NC_CACHE["nc"]


def prep_weights(W_in, W_hh, b_h, W_out, b_out):
    w_in_r = np.ascontiguousarray(
        W_in.reshape(NH, PB, NI, PB).transpose(0, 3, 2, 1)
    )
    w_hh_r = np.ascontiguousarray(
        W_hh.reshape(NH, PB, NH, PB).transpose(0, 3, 2, 1)
    )
    w_out_r = np.ascontiguousarray(
        W_out.reshape(NO, PB, NH, PB).transpose(0, 3, 2, 1)
    )
    bh_r = np.ascontiguousarray(b_h.reshape(NH, PB).T)
    bo_r = np.ascontiguousarray(b_out.reshape(NO, PB).T)
    return w_in_r, w_hh_r, w_out_r, bh_r, bo_r


def make_in_maps(x, h, W_in, W_hh, b_h, W_out, b_out):
    w_in_r, w_hh_r, w_out_r, bh_r, bo_r = prep_weights(W_in, W_hh, b_h, W_out, b_out)
    in_maps = []
    for c in range(NCORES):
        sl = slice(c * BS, (c + 1) * BS)
        in_maps.append(
            {
                "xT": np.ascontiguousarray(x[sl].T),
                "hT": np.ascontiguousarray(h[sl].T),
                "w_in": w_in_r,
                "w_hh": w_hh_r,
                "w_out": w_out_r,
                "bh": bh_r,
                "bo": bo_r,
            }
        )
    return in_maps


def unshard(results):
    y = np.empty((B, O), np.float32)
    h_new = np.empty((B, H), np.float32)
    act64 = np.zeros(H, np.float64)
    rec64 = np.zeros(H, np.float64)
    for c in range(NCORES):
        r = results[c]
        y[c * BS : (c + 1) * BS] = r["yT"].T
        h_new[c * BS : (c + 1) * BS] = r["hnT"].T
        st = r["stats"].astype(np.float64)
        act_pb = st[:, : NH * NBC].reshape(PB, NH, NBC).sum(-1)  # [p, jt]
        rec_pb = (
            st[:, NH * NBC : 2 * NH * NBC] + st[:, 2 * NH * NBC :]
        ).reshape(PB, NH, NBC).sum(-1)  # [p, mt]
        act64 += act_pb.T.reshape(H)
        rec64 += rec_pb.T.reshape(H)
    act_stats = (act64 / B).astype(np.float32)
    rec_stats = (rec64 / B).astype(np.float32)
    return y, h_new, act_stats, rec_stats


def kernel(x, h, W_in, W_hh, b_h, W_out, b_out, **run_kwargs):
    x = np.asarray(x, dtype=np.float32)
    h = np.asarray(h, dtype=np.float32)
    W_in = np.asarray(W_in, dtype=np.float32)
    W_hh = np.asarray(W_hh, dtype=np.float32)
    b_h = np.asarray(b_h, dtype=np.float32)
    W_out = np.asarray(W_out, dtype=np.float32)
    b_out = np.asarray(b_out, dtype=np.float32)

    nc = _get_nc()
    in_maps = make_in_maps(x, h, W_in, W_hh, b_h, W_out, b_out)
    res = run_bass_kernel_spmd(nc, in_maps, core_ids=list(range(NCORES)), **run_kwargs)
    out = unshard(res.results)
    if run_kwargs:
        # expose perf info to a test harness that asked for tracing
        kernel.last_results = res
    return out
